# revision 7
# baseline (speedup 1.0000x reference)
"""Trainium2 Bass kernel for CustomPatchEmbedding (ragged patch gather + two projections).

Strategy (data-parallel over batch, 8 cores x 4 images):
  - Patch pixel rows are gathered straight from HBM images via SWDGE
    indirect DMA. One gather INSTRUCTION covers a whole 128-patch tile
    (2D offset AP [128, nrows], 3D dest AP [128, nrows, px]): the 994ns
    per-instruction SWDGE fixed cost is amortized over 6144 descriptors.
  - Gather indices are computed on-chip from the xy tensors (shift/add on
    DVE) plus small constant offset tables supplied as inputs.
  - Gathered f32 tiles are converted to bf16 on the scalar engine; PE
    transposes 128-feature chunks (bf16, 1 cycle/row) into PSUM slices,
    DVE copies them back as bf16 lhsT tiles, and PE accumulates
    lhsT.T @ W^T (bf16, 1 cycle/row vs 4 for f32) into PSUM.
  - Weights are supplied bf16 from the host and loaded with one packed
    DMA each ([K,D] -> [128, (K/128)*D]); bias is added from a
    partition-replicated f32 tile; results DMA to DRAM in f32.

kernel(**inputs) takes the FULL unsharded inputs and returns (32, 288, 256) f32.
"""
import os
import sys
import numpy as np

sys.path.insert(0, "/opt/trn_rl_repo")

import ml_dtypes
import concourse.bass as bass
import concourse.bacc as bacc
import concourse.mybir as mybir
import concourse.tile as tile
from concourse.masks import make_identity
from concourse.bass_utils import run_bass_kernel_spmd
from contextlib import ExitStack

# Problem constants (hardcoded per spec).
B, C, H, W = 32, 3, 512, 512
FP, CP = 16, 64
NF, NCO = 256, 32
D = 256
NCORES = 8
IPC = B // NCORES              # images per core
CHW = C * H * W                # 786432, per-image flat element count
NFLAT = IPC * CHW              # flat image elements per core
KF = C * FP * FP               # 768  fine features
KC = C * CP * CP               # 12288 coarse features
NROW_F = C * FP                # 48 gather rows per fine patch (c,dy)
NROW_C = C * CP                # 192 gather rows per coarse patch
P = 128

FDT = mybir.dt.float32
BDT = mybir.dt.bfloat16
IDT = mybir.dt.int32

NKF = KF // P                  # 6 fine k-chunks of 128
NKC = KC // P                  # 96 coarse k-chunks of 128
# Coarse gather is split into 4 column-chunks of the [128, 192] index tile,
# keeping each gather instruction at 128*48 = 6144 descriptors.
CJ = 48                        # idx columns per coarse gather chunk
NCHUNK_C = NROW_C // CJ        # 4 chunks
KPC = CJ * CP // P             # k-chunks (of 128) per coarse gather chunk = 24
TPG = 3                        # transposes batched per PSUM tile / DVE copy

# HW SWDGE ucode is "indirect1d": exactly one offset per dest partition, so
# gathers stay per-row. Spread them across the 4 SWDGE queues so descriptor
# generation overlaps. KGATHER=safe pins everything to queue 0.
NQ = 4
MULTI_QUEUE = os.environ.get("KGATHER", "mq") == "mq"
_qctr = [0]


def _indirect_q(nc, out, in_, off):
    """indirect_dma_start clone with round-robin SWDGE queue assignment."""
    g = nc.gpsimd
    q = _qctr[0] % NQ
    _qctr[0] += 1
    if not MULTI_QUEUE or q == 0:
        return g.indirect_dma_start(
            out=out, out_offset=None, in_=in_,
            in_offset=bass.IndirectOffsetOnAxis(ap=off, axis=0),
        )
    out_l = g.lower_ap_dma(out, for_indirect_dma=True)
    in_l = g.lower_ap_dma(in_, for_indirect_dma=True)
    off_l = g.lower_ap_dma(off)
    dyn = mybir.DynamicAccessPatternInfo(
        c=0,
        actual_ap=out.ap,
        indirect_dim_max_index=in_.shape[0],
        offset_expr=[
            mybir.DynamicAccessPatternOffsetExpr(
                coef=1,
                aff_expr=mybir.DynamicAccessPatternOffsetExprAffExpr(
                    kind="IndirectArgId", arg_id=1
                ),
            )
        ],
    )
    in_l[0].dynamic_ap_info = dyn
    return g.add_instruction(
        mybir.InstDMACopy(
            name=nc.get_next_instruction_name(),
            queue=f"qPoolDynamic{q}",
            mode="Copy",
            ins=in_l + off_l,
            outs=out_l,
            oob_is_err=True,
            cce_op=mybir.AluOpType.bypass,
        )
    )


def _emit(nc, tc, t):
    """Emit the per-core Tile program. `t` maps tensor name -> dram handle."""
    with ExitStack() as ctx:
        const = ctx.enter_context(tc.tile_pool(name="const", bufs=1))
        small = ctx.enter_context(tc.tile_pool(name="small", bufs=1))
        gf_pool = ctx.enter_context(tc.tile_pool(name="gf", bufs=3))
        gf16_pool = ctx.enter_context(tc.tile_pool(name="gf16", bufs=3))
        gc_pool = ctx.enter_context(tc.tile_pool(name="gc", bufs=2))
        gc16_pool = ctx.enter_context(tc.tile_pool(name="gc16", bufs=2))
        lt_pool = ctx.enter_context(tc.tile_pool(name="lt", bufs=6))
        ob_pool = ctx.enter_context(tc.tile_pool(name="ob", bufs=3))
        ps_tp = ctx.enter_context(tc.tile_pool(name="ps_tp", bufs=4, space="PSUM"))
        ps_f = ctx.enter_context(tc.tile_pool(name="ps_f", bufs=2, space="PSUM"))
        ps_c = ctx.enter_context(tc.tile_pool(name="ps_c", bufs=1, space="PSUM"))

        # --- weights: packed bf16 loads, resident in SBUF ---
        wf16 = const.tile([P, NKF * D], BDT)
        nc.sync.dma_start(
            wf16[:].rearrange("p (a d) -> p a d", d=D),
            t["wfT"].ap().rearrange("(a p) d -> p a d", p=P),
        )
        wc16 = const.tile([P, NKC * D], BDT)
        nc.sync.dma_start(
            wc16[:].rearrange("p (a d) -> p a d", d=D),
            t["wcT"].ap().rearrange("(a p) d -> p a d", p=P),
        )

        # --- constants ---
        identity = const.tile([P, P], BDT)
        make_identity(nc, identity[:])
        tbl_f = const.tile([P, NROW_F], IDT)
        nc.sync.dma_start(tbl_f[:], t["tbl_f"][:])
        tbl_c = const.tile([P, NROW_C], IDT)
        nc.sync.dma_start(tbl_c[:], t["tbl_c"][:])
        bias_f = const.tile([P, D], FDT)
        nc.sync.dma_start(bias_f[:], t["bias_f"][:])
        bias_c = const.tile([P, D], FDT)
        nc.sync.dma_start(bias_c[:], t["bias_c"][:])

        # --- gather indices ---
        # coarse: one [128, 192] tile; partition p = (img, patch), col j = (c, dy)
        cxy = small.tile([P, 2], IDT)
        nc.sync.dma_start(cxy[:], t["coarse_xy"].ap().rearrange("b n two -> (b n) two"))
        cbase = small.tile([P, 1], IDT)
        nc.vector.tensor_scalar(
            out=cbase[:], in0=cxy[:, 1:2], scalar1=9, scalar2=None,
            op0=mybir.AluOpType.logical_shift_left,
        )
        nc.vector.tensor_tensor(
            out=cbase[:], in0=cbase[:], in1=cxy[:, 0:1], op=mybir.AluOpType.add
        )
        cidx = small.tile([P, NROW_C], IDT)
        nc.vector.tensor_tensor(
            out=cidx[:], in0=tbl_c[:], in1=cbase[:].to_broadcast([P, NROW_C]),
            op=mybir.AluOpType.add,
        )

        # fine: per (img b, half h) a [128, 48] tile
        fidx = []
        for b in range(IPC):
            for h in range(2):
                fxy = small.tile([P, 2], IDT, tag="fxy")
                nc.sync.dma_start(fxy[:], t["fine_xy"][b, h * P:(h + 1) * P, :])
                fb = small.tile([P, 1], IDT, tag="fb")
                nc.vector.tensor_scalar(
                    out=fb[:], in0=fxy[:, 1:2], scalar1=9, scalar2=None,
                    op0=mybir.AluOpType.logical_shift_left,
                )
                nc.vector.tensor_tensor(
                    out=fb[:], in0=fb[:], in1=fxy[:, 0:1], op=mybir.AluOpType.add
                )
                nc.vector.tensor_scalar(
                    out=fb[:], in0=fb[:], scalar1=b * CHW, scalar2=None,
                    op0=mybir.AluOpType.add,
                )
                fi = small.tile([P, NROW_F], IDT, tag=f"fidx{b}{h}")
                nc.vector.tensor_tensor(
                    out=fi[:], in0=tbl_f[:], in1=fb[:].to_broadcast([P, NROW_F]),
                    op=mybir.AluOpType.add,
                )
                fidx.append(fi)

        images = t["images"]
        out = t["out"]

        def gather(gt, idx_ap, ncols, px):
            """Gather ncols*px-wide [128, ncols*px] tile; idx_ap [128, ncols]."""
            for j in range(ncols):
                _indirect_q(
                    nc, gt[:, j * px:(j + 1) * px], images[:], idx_ap[:, j:j + 1]
                )

        def project(gt16, nk, psum, wtile, kbase, nktot):
            """Transpose nk 128-chunks of gt16 and accumulate into psum."""
            for c0 in range(0, nk, TPG):
                cn = min(TPG, nk - c0)
                tp = ps_tp.tile([P, TPG * P], BDT, tag="tp")
                for c in range(c0, c0 + cn):
                    nc.tensor.matmul(
                        out=tp[:, (c - c0) * P:(c - c0 + 1) * P],
                        lhsT=gt16[:, c * P:(c + 1) * P], rhs=identity[:],
                        start=True, stop=True, is_transpose=True,
                        skip_group_check=True,
                    )
                lt = lt_pool.tile([P, TPG * P], BDT, tag="lt")
                nc.vector.tensor_copy(lt[:, :cn * P], tp[:, :cn * P])
                for c in range(c0, c0 + cn):
                    k = kbase + c
                    nc.tensor.matmul(
                        out=psum[:], lhsT=lt[:, (c - c0) * P:(c - c0 + 1) * P],
                        rhs=wtile[:, k * D:(k + 1) * D],
                        start=(k == 0), stop=(k == nktot - 1),
                    )

        # --- fine branch: 8 groups of 128 patches ---
        for g in range(IPC * 2):
            b, h = divmod(g, 2)
            gt = gf_pool.tile([P, KF], FDT)
            gather(gt, fidx[g][:, :], NROW_F, FP)
            gt16 = gf16_pool.tile([P, KF], BDT)
            nc.scalar.copy(gt16[:], gt[:])
            psum = ps_f.tile([P, D], FDT)
            project(gt16, NKF, psum, wf16, 0, NKF)
            ob = ob_pool.tile([P, D], FDT, tag="ob")
            nc.vector.tensor_tensor(
                out=ob[:], in0=psum[:], in1=bias_f[:], op=mybir.AluOpType.add
            )
            nc.sync.dma_start(
                out[b * (NF + NCO) + h * P:b * (NF + NCO) + (h + 1) * P, :], ob[:]
            )

        # --- coarse branch: one group of 128 patches, 4 gather chunks ---
        psum_c = ps_c.tile([P, D], FDT)
        for cc in range(NCHUNK_C):
            gt = gc_pool.tile([P, CJ * CP], FDT)
            gather(gt, cidx[:, cc * CJ:(cc + 1) * CJ], CJ, CP)
            gt16 = gc16_pool.tile([P, CJ * CP], BDT)
            nc.scalar.copy(gt16[:], gt[:])
            project(gt16, KPC, psum_c, wc16, cc * KPC, NKC)
        oc = ob_pool.tile([P, D], FDT, tag="oc")
        nc.vector.tensor_tensor(
            out=oc[:], in0=psum_c[:], in1=bias_c[:], op=mybir.AluOpType.add
        )
        for b in range(IPC):
            nc.sync.dma_start(
                out[b * (NF + NCO) + NF:b * (NF + NCO) + NF + NCO, :],
                oc[b * NCO:(b + 1) * NCO, :],
            )


def build(reps: int = 1):
    nc = bacc.Bacc(
        "TRN2", target_bir_lowering=False, debug=False,
        num_swdge_queues=NQ if MULTI_QUEUE else 1,
    )
    t = {
        "images": nc.dram_tensor("images", [NFLAT, 1], FDT, kind="ExternalInput"),
        "fine_xy": nc.dram_tensor("fine_xy", [IPC, NF, 2], IDT, kind="ExternalInput"),
        "coarse_xy": nc.dram_tensor("coarse_xy", [IPC, NCO, 2], IDT, kind="ExternalInput"),
        "wfT": nc.dram_tensor("wfT", [KF, D], BDT, kind="ExternalInput"),
        "wcT": nc.dram_tensor("wcT", [KC, D], BDT, kind="ExternalInput"),
        "bias_f": nc.dram_tensor("bias_f", [P, D], FDT, kind="ExternalInput"),
        "bias_c": nc.dram_tensor("bias_c", [P, D], FDT, kind="ExternalInput"),
        "tbl_f": nc.dram_tensor("tbl_f", [P, NROW_F], IDT, kind="ExternalInput"),
        "tbl_c": nc.dram_tensor("tbl_c", [P, NROW_C], IDT, kind="ExternalInput"),
        "out": nc.dram_tensor("out", [IPC * (NF + NCO), D], FDT, kind="ExternalOutput"),
    }
    with tile.TileContext(nc) as tc:
        for _ in range(reps):
            _emit(nc, tc, t)
    nc.compile()
    return nc


def host_tables():
    jf = np.arange(NROW_F)
    tbl_f = ((jf // FP) * H * W + (jf % FP) * W).astype(np.int32)
    tbl_f = np.repeat(tbl_f[None, :], P, axis=0)
    pc = np.arange(P)[:, None]
    jc = np.arange(NROW_C)[None, :]
    tbl_c = ((pc // NCO) * CHW + (jc // CP) * H * W + (jc % CP) * W).astype(np.int32)
    tbl_c = np.ascontiguousarray(tbl_c)
    return tbl_f, tbl_c


def make_in_maps(images, W_fine, b_fine, W_coarse, b_coarse, fine_xy, coarse_xy):
    images = np.asarray(images, dtype=np.float32)
    fine_xy = np.asarray(fine_xy, dtype=np.int32)
    coarse_xy = np.asarray(coarse_xy, dtype=np.int32)
    wfT = np.ascontiguousarray(
        np.asarray(W_fine, dtype=np.float32).T.astype(ml_dtypes.bfloat16)
    )
    wcT = np.ascontiguousarray(
        np.asarray(W_coarse, dtype=np.float32).T.astype(ml_dtypes.bfloat16)
    )
    bias_f = np.ascontiguousarray(np.repeat(np.asarray(b_fine, np.float32)[None, :], P, axis=0))
    bias_c = np.ascontiguousarray(np.repeat(np.asarray(b_coarse, np.float32)[None, :], P, axis=0))
    tbl_f, tbl_c = host_tables()
    in_maps = []
    for c in range(NCORES):
        sl = slice(c * IPC, (c + 1) * IPC)
        in_maps.append({
            "images": np.ascontiguousarray(images[sl]).reshape(NFLAT, 1),
            "fine_xy": np.ascontiguousarray(fine_xy[sl]),
            "coarse_xy": np.ascontiguousarray(coarse_xy[sl]),
            "wfT": wfT, "wcT": wcT,
            "bias_f": bias_f, "bias_c": bias_c,
            "tbl_f": tbl_f, "tbl_c": tbl_c,
        })
    return in_maps


_NC_CACHE = []


def _get_nc():
    if not _NC_CACHE:
        _NC_CACHE.append(build())
    return _NC_CACHE[0]


def run(inputs: dict, trace: bool = False):
    nc = _get_nc()
    in_maps = make_in_maps(**inputs)
    res = run_bass_kernel_spmd(nc, in_maps, list(range(NCORES)), trace=trace)
    outs = [
        np.asarray(res.results[c]["out"]).reshape(IPC, NF + NCO, D)
        for c in range(NCORES)
    ]
    return np.concatenate(outs, axis=0), res


def kernel(**inputs) -> np.ndarray:
    out, _ = run(inputs, trace=False)
    return out


# revision 17
# speedup vs baseline: 1.5571x; 1.5571x over previous
"""Trainium2 Bass kernel for CustomPatchEmbedding (ragged patch gather + two projections).

Strategy (data-parallel over batch, 8 cores x 4 images):
  - Patch pixel rows are gathered straight from HBM images via SWDGE
    indirect DMA. One gather INSTRUCTION covers a whole 128-patch tile
    (2D offset AP [128, nrows], 3D dest AP [128, nrows, px]): the 994ns
    per-instruction SWDGE fixed cost is amortized over 6144 descriptors.
  - Gather indices are computed on-chip from the xy tensors (shift/add on
    DVE) plus small constant offset tables supplied as inputs.
  - Gathered f32 tiles are converted to bf16 on the scalar engine; PE
    transposes 128-feature chunks (bf16, 1 cycle/row) into PSUM slices,
    DVE copies them back as bf16 lhsT tiles, and PE accumulates
    lhsT.T @ W^T (bf16, 1 cycle/row vs 4 for f32) into PSUM.
  - Weights are supplied bf16 from the host and loaded with one packed
    DMA each ([K,D] -> [128, (K/128)*D]); bias is added from a
    partition-replicated f32 tile; results DMA to DRAM in f32.

kernel(**inputs) takes the FULL unsharded inputs and returns (32, 288, 256) f32.
"""
import os
import sys
import numpy as np

sys.path.insert(0, "/opt/trn_rl_repo")

import ml_dtypes
import concourse.bass as bass
import concourse.bacc as bacc
import concourse.mybir as mybir
import concourse.tile as tile
from concourse.masks import make_identity
from concourse.bass_utils import run_bass_kernel_spmd
from contextlib import ExitStack

# Problem constants (hardcoded per spec).
B, C, H, W = 32, 3, 512, 512
FP, CP = 16, 64
NF, NCO = 256, 32
D = 256
NCORES = 8
IPC = B // NCORES              # images per core
CHW = C * H * W                # 786432, per-image flat element count
NFLAT = IPC * CHW              # flat image elements per core
KF = C * FP * FP               # 768  fine features
KC = C * CP * CP               # 12288 coarse features
NROW_F = C * FP                # 48 gather rows per fine patch (c,dy)
NROW_C = C * CP                # 192 gather rows per coarse patch
P = 128

FDT = mybir.dt.float32
BDT = mybir.dt.bfloat16
IDT = mybir.dt.int32

NKF = KF // P                  # 6 fine k-chunks of 128
NKC = KC // P                  # 96 coarse k-chunks of 128
# Coarse gather is split into column-chunks of the [128, 192] index tile.
CJ = 24                        # idx columns per coarse gather chunk
NCHUNK_C = NROW_C // CJ        # 8 chunks
KPC = CJ * CP // P             # k-chunks (of 128) per coarse gather chunk = 12
TPG = 3                        # transposes batched per PSUM tile / DVE copy

# HW SWDGE ucode is "indirect1d": exactly one offset per dest partition, so
# gathers stay per-row. Spread them across the 4 SWDGE queues so descriptor
# generation overlaps. KGATHER=safe pins everything to queue 0.
NQ = 4
MULTI_QUEUE = os.environ.get("KGATHER", "sq") == "mq"
RPAD = FP * W                  # fine row-block gather span (16 image rows)
NFLAT_PAD = NFLAT + RPAD
_qctr = [0]


def _indirect_q(nc, out, in_, off):
    """indirect_dma_start clone with round-robin SWDGE queue assignment."""
    g = nc.gpsimd
    q = _qctr[0] % NQ
    _qctr[0] += 1
    if not MULTI_QUEUE or q == 0:
        return g.indirect_dma_start(
            out=out, out_offset=None, in_=in_,
            in_offset=bass.IndirectOffsetOnAxis(ap=off, axis=0),
        )
    out_l = g.lower_ap_dma(out, for_indirect_dma=True)
    in_l = g.lower_ap_dma(in_, for_indirect_dma=True)
    off_l = g.lower_ap_dma(off)
    dyn = mybir.DynamicAccessPatternInfo(
        c=0,
        actual_ap=out.ap,
        indirect_dim_max_index=in_.shape[0],
        offset_expr=[
            mybir.DynamicAccessPatternOffsetExpr(
                coef=1,
                aff_expr=mybir.DynamicAccessPatternOffsetExprAffExpr(
                    kind="IndirectArgId", arg_id=1
                ),
            )
        ],
    )
    in_l[0].dynamic_ap_info = dyn
    return g.add_instruction(
        mybir.InstDMACopy(
            name=nc.get_next_instruction_name(),
            queue=f"qPoolDynamic{q}",
            mode="Copy",
            ins=in_l + off_l,
            outs=out_l,
            oob_is_err=True,
            cce_op=mybir.AluOpType.bypass,
        )
    )


def _emit(nc, tc, t):
    """Emit the per-core Tile program. `t` maps tensor name -> dram handle."""
    with ExitStack() as ctx:
        const = ctx.enter_context(tc.tile_pool(name="const", bufs=1))
        small = ctx.enter_context(tc.tile_pool(name="small", bufs=1))
        gb_pool = ctx.enter_context(tc.tile_pool(name="gb", bufs=2))
        gf16_pool = ctx.enter_context(tc.tile_pool(name="gf16", bufs=3))
        gc_pool = ctx.enter_context(tc.tile_pool(name="gc", bufs=2))
        gc16_pool = ctx.enter_context(tc.tile_pool(name="gc16", bufs=2))
        lt_pool = ctx.enter_context(tc.tile_pool(name="lt", bufs=6))
        ob_pool = ctx.enter_context(tc.tile_pool(name="ob", bufs=3))
        ps_tp = ctx.enter_context(tc.tile_pool(name="ps_tp", bufs=4, space="PSUM"))
        ps_f = ctx.enter_context(tc.tile_pool(name="ps_f", bufs=2, space="PSUM"))
        ps_c = ctx.enter_context(tc.tile_pool(name="ps_c", bufs=1, space="PSUM"))

        # --- weights: packed bf16 loads, resident in SBUF ---
        wf16 = const.tile([P, NKF * D], BDT)
        nc.sync.dma_start(
            wf16[:].rearrange("p (a d) -> p a d", d=D),
            t["wfT"].ap().rearrange("(a p) d -> p a d", p=P),
        )
        wc16 = const.tile([P, NKC * D], BDT)
        nc.sync.dma_start(
            wc16[:].rearrange("p (a d) -> p a d", d=D),
            t["wcT"].ap().rearrange("(a p) d -> p a d", p=P),
        )

        # --- constants ---
        identity = const.tile([P, P], BDT)
        make_identity(nc, identity[:])
        tbl_fc = const.tile([P, C], IDT)
        nc.sync.dma_start(tbl_fc[:], t["tbl_fc"][:])
        tbl_c = const.tile([P, NROW_C], IDT)
        nc.sync.dma_start(tbl_c[:], t["tbl_c"][:])
        bias_f = const.tile([P, D], FDT)
        nc.sync.dma_start(bias_f[:], t["bias_f"][:])
        bias_c = const.tile([P, D], FDT)
        nc.sync.dma_start(bias_c[:], t["bias_c"][:])

        # --- gather indices ---
        # coarse: one [128, 192] tile; partition p = (img, patch), col j = (c, dy)
        cxy = small.tile([P, 2], IDT)
        nc.sync.dma_start(cxy[:], t["coarse_xy"].ap().rearrange("b n two -> (b n) two"))
        cbase = small.tile([P, 1], IDT)
        nc.vector.tensor_scalar(
            out=cbase[:], in0=cxy[:, 1:2], scalar1=9, scalar2=None,
            op0=mybir.AluOpType.logical_shift_left,
        )
        nc.vector.tensor_tensor(
            out=cbase[:], in0=cbase[:], in1=cxy[:, 0:1], op=mybir.AluOpType.add
        )
        cidx = small.tile([P, NROW_C], IDT)
        nc.vector.tensor_tensor(
            out=cidx[:], in0=tbl_c[:], in1=cbase[:].to_broadcast([P, NROW_C]),
            op=mybir.AluOpType.add,
        )

        # fine: per (img b, half h) a [128, 3] tile of per-channel block bases
        # (b*CHW + c*HW + y*W + x) -- one row-block gather per channel.
        fidx = []
        for b in range(IPC):
            for h in range(2):
                fxy = small.tile([P, 2], IDT, tag="fxy")
                nc.sync.dma_start(fxy[:], t["fine_xy"][b, h * P:(h + 1) * P, :])
                fb = small.tile([P, 1], IDT, tag="fb")
                nc.vector.tensor_scalar(
                    out=fb[:], in0=fxy[:, 1:2], scalar1=9, scalar2=None,
                    op0=mybir.AluOpType.logical_shift_left,
                )
                nc.vector.tensor_tensor(
                    out=fb[:], in0=fb[:], in1=fxy[:, 0:1], op=mybir.AluOpType.add
                )
                nc.vector.tensor_scalar(
                    out=fb[:], in0=fb[:], scalar1=b * CHW, scalar2=None,
                    op0=mybir.AluOpType.add,
                )
                fi = small.tile([P, C], IDT, tag=f"fidx{b}{h}")
                nc.vector.tensor_tensor(
                    out=fi[:], in0=tbl_fc[:], in1=fb[:].to_broadcast([P, C]),
                    op=mybir.AluOpType.add,
                )
                fidx.append(fi)

        images = t["images"]
        out = t["out"]

        def gather(gt, idx_ap, ncols, px):
            """Gather ncols*px-wide [128, ncols*px] tile; idx_ap [128, ncols]."""
            for j in range(ncols):
                _indirect_q(
                    nc, gt[:, j * px:(j + 1) * px], images[:], idx_ap[:, j:j + 1]
                )

        def project(gt16, nk, psum, wtile, kbase, nktot):
            """Transpose nk 128-chunks of gt16 and accumulate into psum."""
            for c0 in range(0, nk, TPG):
                cn = min(TPG, nk - c0)
                tp = ps_tp.tile([P, TPG * P], BDT, tag="tp")
                for c in range(c0, c0 + cn):
                    nc.tensor.matmul(
                        out=tp[:, (c - c0) * P:(c - c0 + 1) * P],
                        lhsT=gt16[:, c * P:(c + 1) * P], rhs=identity[:],
                        start=True, stop=True, is_transpose=True,
                        skip_group_check=True,
                    )
                lt = lt_pool.tile([P, TPG * P], BDT, tag="lt")
                nc.vector.tensor_copy(lt[:, :cn * P], tp[:, :cn * P])
                for c in range(c0, c0 + cn):
                    k = kbase + c
                    nc.tensor.matmul(
                        out=psum[:], lhsT=lt[:, (c - c0) * P:(c - c0 + 1) * P],
                        rhs=wtile[:, k * D:(k + 1) * D],
                        start=(k == 0), stop=(k == nktot - 1),
                    )

        # --- fine branch: 8 groups of 128 patches ---
        # Per (group, channel): ONE indirect DMA pulls 16 full image rows
        # (RPAD elems) per patch; the 16x16 patch sits at static columns
        # dy*W .. dy*W+15, extracted by a strided scalar-engine copy that
        # also converts f32 -> bf16.
        for g in range(IPC * 2):
            b, h = divmod(g, 2)
            gt16 = gf16_pool.tile([P, KF], BDT)
            for c in range(C):
                gtb = gb_pool.tile([P, RPAD], FDT, tag="gtb")
                _indirect_q(nc, gtb[:], images[:], fidx[g][:, c:c + 1])
                nc.scalar.copy(
                    gt16[:, c * FP * FP:(c + 1) * FP * FP].rearrange(
                        "p (r k) -> p r k", k=FP
                    ),
                    gtb[:].rearrange("p (r q) -> p r q", q=W)[:, :, :FP],
                )
            psum = ps_f.tile([P, D], FDT)
            project(gt16, NKF, psum, wf16, 0, NKF)
            ob = ob_pool.tile([P, D], FDT, tag="ob")
            nc.vector.tensor_tensor(
                out=ob[:], in0=psum[:], in1=bias_f[:], op=mybir.AluOpType.add
            )
            nc.sync.dma_start(
                out[b * (NF + NCO) + h * P:b * (NF + NCO) + (h + 1) * P, :], ob[:]
            )

        # --- coarse branch: one group of 128 patches, 4 gather chunks ---
        psum_c = ps_c.tile([P, D], FDT)
        for cc in range(NCHUNK_C):
            gt = gc_pool.tile([P, CJ * CP], FDT)
            gather(gt, cidx[:, cc * CJ:(cc + 1) * CJ], CJ, CP)
            gt16 = gc16_pool.tile([P, CJ * CP], BDT)
            nc.scalar.copy(gt16[:], gt[:])
            project(gt16, KPC, psum_c, wc16, cc * KPC, NKC)
        oc = ob_pool.tile([P, D], FDT, tag="oc")
        nc.vector.tensor_tensor(
            out=oc[:], in0=psum_c[:], in1=bias_c[:], op=mybir.AluOpType.add
        )
        for b in range(IPC):
            nc.sync.dma_start(
                out[b * (NF + NCO) + NF:b * (NF + NCO) + NF + NCO, :],
                oc[b * NCO:(b + 1) * NCO, :],
            )


def build(reps: int = 1):
    nc = bacc.Bacc(
        "TRN2", target_bir_lowering=False, debug=False,
        num_swdge_queues=NQ if MULTI_QUEUE else 1,
    )
    t = {
        "images": nc.dram_tensor("images", [NFLAT_PAD, 1], FDT, kind="ExternalInput"),
        "fine_xy": nc.dram_tensor("fine_xy", [IPC, NF, 2], IDT, kind="ExternalInput"),
        "coarse_xy": nc.dram_tensor("coarse_xy", [IPC, NCO, 2], IDT, kind="ExternalInput"),
        "wfT": nc.dram_tensor("wfT", [KF, D], BDT, kind="ExternalInput"),
        "wcT": nc.dram_tensor("wcT", [KC, D], BDT, kind="ExternalInput"),
        "bias_f": nc.dram_tensor("bias_f", [P, D], FDT, kind="ExternalInput"),
        "bias_c": nc.dram_tensor("bias_c", [P, D], FDT, kind="ExternalInput"),
        "tbl_fc": nc.dram_tensor("tbl_fc", [P, C], IDT, kind="ExternalInput"),
        "tbl_c": nc.dram_tensor("tbl_c", [P, NROW_C], IDT, kind="ExternalInput"),
        "out": nc.dram_tensor("out", [IPC * (NF + NCO), D], FDT, kind="ExternalOutput"),
    }
    with tile.TileContext(nc) as tc:
        for _ in range(reps):
            _emit(nc, tc, t)
    nc.compile()
    return nc


def host_tables():
    tbl_fc = np.repeat(
        (np.arange(C) * H * W).astype(np.int32)[None, :], P, axis=0
    )
    tbl_fc = np.ascontiguousarray(tbl_fc)
    pc = np.arange(P)[:, None]
    jc = np.arange(NROW_C)[None, :]
    tbl_c = ((pc // NCO) * CHW + (jc // CP) * H * W + (jc % CP) * W).astype(np.int32)
    tbl_c = np.ascontiguousarray(tbl_c)
    return tbl_fc, tbl_c


def make_in_maps(images, W_fine, b_fine, W_coarse, b_coarse, fine_xy, coarse_xy):
    images = np.asarray(images, dtype=np.float32)
    fine_xy = np.asarray(fine_xy, dtype=np.int32)
    coarse_xy = np.asarray(coarse_xy, dtype=np.int32)
    wfT = np.ascontiguousarray(
        np.asarray(W_fine, dtype=np.float32).T.astype(ml_dtypes.bfloat16)
    )
    wcT = np.ascontiguousarray(
        np.asarray(W_coarse, dtype=np.float32).T.astype(ml_dtypes.bfloat16)
    )
    bias_f = np.ascontiguousarray(np.repeat(np.asarray(b_fine, np.float32)[None, :], P, axis=0))
    bias_c = np.ascontiguousarray(np.repeat(np.asarray(b_coarse, np.float32)[None, :], P, axis=0))
    tbl_fc, tbl_c = host_tables()
    pad = np.zeros((RPAD, 1), np.float32)
    in_maps = []
    for c in range(NCORES):
        sl = slice(c * IPC, (c + 1) * IPC)
        img = np.concatenate(
            [np.ascontiguousarray(images[sl]).reshape(NFLAT, 1), pad], axis=0
        )
        in_maps.append({
            "images": img,
            "fine_xy": np.ascontiguousarray(fine_xy[sl]),
            "coarse_xy": np.ascontiguousarray(coarse_xy[sl]),
            "wfT": wfT, "wcT": wcT,
            "bias_f": bias_f, "bias_c": bias_c,
            "tbl_fc": tbl_fc, "tbl_c": tbl_c,
        })
    return in_maps


_NC_CACHE = []


def _get_nc():
    if not _NC_CACHE:
        _NC_CACHE.append(build())
    return _NC_CACHE[0]


def run(inputs: dict, trace: bool = False):
    nc = _get_nc()
    in_maps = make_in_maps(**inputs)
    res = run_bass_kernel_spmd(nc, in_maps, list(range(NCORES)), trace=trace)
    outs = [
        np.asarray(res.results[c]["out"]).reshape(IPC, NF + NCO, D)
        for c in range(NCORES)
    ]
    return np.concatenate(outs, axis=0), res


def kernel(**inputs) -> np.ndarray:
    out, _ = run(inputs, trace=False)
    return out
